# revision 26
# baseline (speedup 1.0000x reference)
"""Sliding-window causal GQA self-attention (B=2, T=2048, 16 q-heads, 4 kv-heads,
head_dim=128, window=1024) on 8 trn2 NeuronCores.

Sharding: core = (batch b, kv-group g) -> 4 query heads + 1 kv head, full T.
Wo is row-parallel; each core emits a [T, 2048] partial that the host sums per
batch (the unshard step for the row-parallel layout).

All matmul operands are bf16 (1 cycle/row on PE at any free size; halves DMA
traffic vs fp32); PSUM accumulation stays fp32. The whole program uses a single
activation-function table set (natural_log_exp_and_others: exp/ln/copy/square),
so no LoadActFuncSet reloads: the RMS rsqrt is computed as exp(-0.5*ln(ms+eps)).

Device dataflow:
  phase 1 (per 256-token chunk): qT/kT/vT projections (bf16), squares on ACT
           straight from PSUM, per-pair ones-matmul partition-sum, Ln+Exp rms
           factors, RoPE (half-swap via PSUM->SBUF DMA + [c;c], [s;-s] tables),
           gate sigmoid via Exp, V^T -> natural V via DMA transpose (bf16).
  phase 2: S^T = K^T.T @ Q^T per 128-key block x 256-query super (2 heads);
           ACT exp (scale fused) -> bf16; 0/1 triangle masks; PV + all-ones
           rowsum accumulated in PSUM; normalize on evacuation into yT (bf16).
           Far-edge key block computes only the live low query half.
           Phase-3 output matmuls for query-super qs-1 are interleaved between
           head-pairs to fill PE bubbles.
  phase 3 (interleaved): out[t, o] = sum_h yT_h^T @ Wo_h, Wo fully prefetched.
"""

import numpy as np

B, T, E = 2, 2048, 2048
NH, NKV, HD = 16, 4, 128
GATE_C = 32
WIN = 1024
EPS = 1e-6
NE = E // 128          # 16 contraction chunks
TC = 256               # phase-1 token chunk (= q-super width)
NTC = T // TC          # 8
NKB = T // 128         # 16 key blocks
SCALE = 1.0 / np.sqrt(HD)

_CACHE = {}


def _build_program():
    import concourse.bacc as bacc
    import concourse.mybir as mybir
    import concourse.tile as tile
    from concourse import bass_isa

    F32, BF16 = mybir.dt.float32, mybir.dt.bfloat16
    AF = mybir.ActivationFunctionType
    OP = mybir.AluOpType

    nc = bacc.Bacc("TRN2", target_bir_lowering=False, debug=False, num_devices=8)

    xT = nc.dram_tensor("xT", [E, T], BF16, kind="ExternalInput")
    veT = nc.dram_tensor("veT", [HD, T], BF16, kind="ExternalInput")
    crep = nc.dram_tensor("crep", [128, T], BF16, kind="ExternalInput")
    ssgn = nc.dram_tensor("ssgn", [128, T], BF16, kind="ExternalInput")
    wq = nc.dram_tensor("wq", [E, 512], BF16, kind="ExternalInput")
    wk = nc.dram_tensor("wk", [E, HD], BF16, kind="ExternalInput")
    wv = nc.dram_tensor("wv", [E, HD], BF16, kind="ExternalInput")
    wg = nc.dram_tensor("wg", [GATE_C, 128], BF16, kind="ExternalInput")
    wo = nc.dram_tensor("wo", [512, E], BF16, kind="ExternalInput")
    m_in = nc.dram_tensor("m_in", [4, 128, 512], BF16, kind="ExternalInput")
    mn_in = nc.dram_tensor("mn_in", [2, 128, 256], BF16, kind="ExternalInput")
    ones_in = nc.dram_tensor("ones_in", [128, 128], BF16, kind="ExternalInput")
    out = nc.dram_tensor("out", [T, E], BF16, kind="ExternalOutput")

    xT_r = xT.rearrange("(e k) t -> k e t", k=128)
    wq_r = wq.rearrange("(e k) d -> k e d", k=128)
    wk_r = wk.rearrange("(e k) d -> k e d", k=128)
    wv_r = wv.rearrange("(e k) d -> k e d", k=128)
    wo_r = wo.rearrange("(h d) o -> d h o", d=128)

    with tile.TileContext(nc) as tc:
        from contextlib import ExitStack
        with ExitStack() as ctx:
            cst = ctx.enter_context(tc.tile_pool(name="cst", bufs=1))
            wts = ctx.enter_context(tc.tile_pool(name="wts", bufs=1))
            xtp = ctx.enter_context(tc.tile_pool(name="xtp", bufs=3))
            res = ctx.enter_context(tc.tile_pool(name="res", bufs=1))
            qrp = ctx.enter_context(tc.tile_pool(name="qrp", bufs=4))
            wk1 = ctx.enter_context(tc.tile_pool(name="wk1", bufs=4))
            wk2 = ctx.enter_context(tc.tile_pool(name="wk2", bufs=2))
            ptp = ctx.enter_context(tc.tile_pool(name="ptp", bufs=4))
            stg = ctx.enter_context(tc.tile_pool(name="stg", bufs=4))
            p_q = ctx.enter_context(tc.tile_pool(name="p_q", bufs=2, space="PSUM"))
            p_s = ctx.enter_context(tc.tile_pool(name="p_s", bufs=3, space="PSUM"))
            p_or = ctx.enter_context(tc.tile_pool(name="p_or", bufs=3, space="PSUM"))

            # ---- tiny constants + chunk-0 / weight stream, round-robin by
            # e-group so the projection chains can start as data arrives ----
            ones_sb = cst.tile([128, 128], BF16, tag="ones")
            eps_sb = cst.tile([128, 1], F32, tag="eps")
            wg_sb = wts.tile([GATE_C, 128], BF16, tag="wg")
            nc.sync.dma_start(out=ones_sb, in_=ones_in[:])
            nc.vector.memset(eps_sb, EPS)
            nc.sync.dma_start(out=wg_sb, in_=wg[:])

            # Pin the act table to natural_log_exp_and_others (set 6): every
            # activation in this program (Exp, Ln, Square, Copy, Identity) is
            # in it, so the auto-insertion pass never needs another load.
            nc.scalar.add_instruction(mybir.InstLoadActFuncSet(
                name=nc.get_next_instruction_name(), ins=[], outs=[],
                act_func_set_id=6))

            # gate input first: it's tiny and gives PE work immediately
            xg_sb = cst.tile([GATE_C, T], BF16, tag="xg")
            nc.sync.dma_start(out=xg_sb, in_=xT[0:GATE_C, :])

            xt0 = xtp.tile([128, NE, TC], BF16, tag="xt")
            wq_sb = wts.tile([128, NE, 512], BF16, tag="wq")
            wk_sb = wts.tile([128, NE, HD], BF16, tag="wk")
            wv_sb = wts.tile([128, NE, HD], BF16, tag="wv")
            for e4 in range(4):
                sl = slice(e4 * 4, (e4 + 1) * 4)
                nc.sync.dma_start(out=xt0[:, sl, :], in_=xT_r[:, sl, 0:TC])
                nc.sync.dma_start(out=wk_sb[:, sl, :], in_=wk_r[:, sl, :])
                nc.sync.dma_start(out=wq_sb[:, sl, :], in_=wq_r[:, sl, :])
            for e4 in range(4):
                sl = slice(e4 * 4, (e4 + 1) * 4)
                nc.sync.dma_start(out=wv_sb[:, sl, :], in_=wv_r[:, sl, :])

            # rope/ve tables resident for the whole run
            crep_sb = cst.tile([128, T], BF16, tag="crep")
            ssgn_sb = cst.tile([128, T], BF16, tag="ssgn")
            veT_sb = cst.tile([HD, T], BF16, tag="veT")
            nc.sync.dma_start(out=crep_sb, in_=crep[:])
            nc.sync.dma_start(out=ssgn_sb, in_=ssgn[:])
            nc.sync.dma_start(out=veT_sb, in_=veT[:])

            masks_sb = cst.tile([128, 4, 512], BF16, tag="masks")
            masksn_sb = cst.tile([128, 2, 256], BF16, tag="masksn")
            wo_sb = wts.tile([128, 4, E], BF16, tag="wo")

            # ---- persistent results ----
            qT_sb = res.tile([128, 4, T], BF16, tag="qT")
            kT_sb = res.tile([128, T], BF16, tag="kT")
            yT_sb = res.tile([128, 4, T], BF16, tag="yT")
            vn_sb = res.tile([128, NKB, HD], BF16, tag="vn")

            # ================= phase 0: all gate sigmoids ==================
            # One Exp act-table period at program start; phase 1 then runs on
            # the Sqrt table only and phase 2/3 on Exp only (3 loads total).
            # g = 1/(1+exp(-u)); the 2x is folded into the v STT later.
            g_all = cst.tile([128, T], F32, tag="gall")
            for gs in range(4):
                sl = slice(gs * 512, (gs + 1) * 512)
                gp = p_s.tile([128, 512], F32, tag="s")
                nc.tensor.matmul(gp, wg_sb, xg_sb[:, sl], start=True, stop=True)
                nc.scalar.activation(g_all[:, sl], gp, AF.Exp, scale=-1.0)
                nc.vector.tensor_scalar_add(g_all[:, sl], g_all[:, sl], 1.0)
                nc.vector.reciprocal(g_all[:, sl], g_all[:, sl])

            # ================= phase 1 =====================================
            _CHUNK_XT = {}
            for tcix in range(NTC):
                ts = tcix * TC
                if tcix == 0:
                    xt = xt0
                    xt_next = None
                else:
                    xt = _CHUNK_XT[tcix]
                    xt_next = None
                if tcix + 1 < NTC:
                    xt_next = xtp.tile([128, NE, TC], BF16, tag="xt")
                    _CHUNK_XT[tcix + 1] = xt_next
                c_sl = crep_sb[:, ts:ts + TC]
                s_sl = ssgn_sb[:, ts:ts + TC]
                ve_sl = veT_sb[:, ts:ts + TC]
                g_rep = g_all[:, ts:ts + TC]

                def prefetch(part):
                    # spread next-chunk x DMA through this chunk so it never
                    # blocks latency-critical small transfers on the queue
                    if xt_next is not None:
                        sl = slice(part * 4, (part + 1) * 4)
                        nc.sync.dma_start(
                            out=xt_next[:, sl, :],
                            in_=xT_r[:, sl, (ts + TC):(ts + 2 * TC)])

                # k first so attention's S matmuls unblock as early as
                # possible; v mid-chunk so vn is ready before the head tail
                srcs = [("k", 0), ("q", 0), ("q", 1), ("q", 2), ("q", 3)]
                chunk_qraws = []
                sq_pair = None
                rr_pair = None
                for i, (kind, h) in enumerate(srcs):
                    ps = p_q.tile([128, TC], F32, tag="q")
                    w_sb = wq_sb if kind == "q" else wk_sb
                    for e in range(NE):
                        lhs = w_sb[:, e, h * 128:(h + 1) * 128] if kind == "q" else w_sb[:, e, :]
                        nc.tensor.matmul(ps, lhs, xt[:, e, :],
                                         start=(e == 0), stop=(e == NE - 1))
                    half = i % 2
                    if half == 0:
                        sq_pair = wk1.tile([128, 512], BF16, tag="sq")
                        rr_pair = wk2.tile([128, 512], F32, tag="rrms")
                    nc.scalar.activation(sq_pair[:, half * TC:(half + 1) * TC],
                                         ps, AF.Square)
                    # rotate-half partner via DVE partition shuffle (groups of
                    # 4 partitions; swapping halves is an involution so the
                    # mask direction is irrelevant)
                    qsw = wk1.tile([128, TC], F32, tag="qsw")
                    nc.vector.stream_shuffle(qsw, ps,
                                             list(range(16, 32)) + list(range(16)))
                    chunk_qraws.append((ps, qsw))
                    if half == 1 or i == 4:
                        wd = 512 if half == 1 else 256
                        ss_sb = wk2.tile([128, 512], F32, tag="ssr")
                        nc.gpsimd.partition_all_reduce(
                            ss_sb[:, 0:wd], sq_pair[:, 0:wd], channels=128,
                            reduce_op=bass_isa.ReduceOp.add)
                        lt = wk1.tile([128, 512], F32, tag="lt")
                        nc.scalar.activation(lt[:, 0:wd], ss_sb[:, 0:wd],
                                             AF.Ln, bias=eps_sb, scale=1.0 / HD)
                        nc.scalar.activation(rr_pair[:, 0:wd], lt[:, 0:wd],
                                             AF.Exp, scale=-0.5)
                        done = [i - 1, i] if half == 1 else [i]
                        for ii in done:
                            kind2, h2 = srcs[ii]
                            qraw2, qsw2 = chunk_qraws[ii]
                            rrms = rr_pair[:, (ii % 2) * TC:(ii % 2 + 1) * TC]
                            tA = wk1.tile([128, TC], F32, tag="tA")
                            tB = wk1.tile([128, TC], F32, tag="tB")
                            nc.vector.tensor_tensor(tA, qraw2, c_sl, OP.mult)
                            nc.gpsimd.tensor_tensor(tB, qsw2, s_sl, OP.mult)
                            nc.vector.tensor_add(tA, tA, tB)
                            dest = (qT_sb[:, h2, ts:ts + TC] if kind2 == "q"
                                    else kT_sb[:, ts:ts + TC])
                            nc.vector.tensor_mul(dest, tA, rrms)
                        prefetch(i // 2)

                    if i == 2:
                        # v chain mid-chunk: projection + gated ve; natural
                        # layout via DMA transpose
                        ps_v = p_q.tile([128, TC], F32, tag="q")
                        for e in range(NE):
                            nc.tensor.matmul(ps_v, wv_sb[:, e, :], xt[:, e, :],
                                             start=(e == 0), stop=(e == NE - 1))
                        tv = wk1.tile([128, TC], F32, tag="tA")
                        nc.gpsimd.tensor_tensor(tv, ve_sl, g_rep, OP.mult)
                        vt = wk1.tile([128, TC], BF16, tag="tB")
                        nc.vector.scalar_tensor_tensor(vt, tv, 2.0, ps_v,
                                                       OP.mult, OP.add)
                        for tb in range(TC // 128):
                            nc.sync.dma_start_transpose(
                                out=vn_sb[:, tcix * 2 + tb, :],
                                in_=vt[:, tb * 128:(tb + 1) * 128])

                prefetch(3)
                if tcix == 1:
                    # phase-2 masks: needed once attention for qs=0 hoists in
                    nc.sync.dma_start(out=masks_sb,
                                      in_=m_in.rearrange("m p f -> p m f"))
                    nc.sync.dma_start(out=masksn_sb,
                                      in_=mn_in.rearrange("m p f -> p m f"))
                if tcix == 3:
                    # full Wo prefetch (bf16, 2 MiB); first used by emit_out(0)
                    nc.sync.dma_start(out=wo_sb, in_=wo_r)

            # ============ phase 2 + interleaved phase 3 ====================
            def emit_attn(hp, qs):
                h2 = slice(2 * hp, 2 * hp + 2)
                q0 = qs * TC
                kb0 = max(0, 2 * qs - 8)
                kb1 = 2 * qs + 2
                far = qs >= 4  # far window edge exists -> kb0 is half-live
                o_ps = p_or.tile([128, 512], F32, tag="or")
                r_ps = p_or.tile([128, 512], F32, tag="or")
                o_v = o_ps.rearrange("p (h q) -> p h q", h=2)
                r_v = r_ps.rearrange("p (h q) -> p h q", h=2)
                kbs = list(range(kb0, kb1))
                if far:
                    # kb0 only touches the low query half; emit kb0+1 first so
                    # it opens (start=True) the full-width PSUM accumulation.
                    kbs[0], kbs[1] = kbs[1], kbs[0]
                first = kbs[0]
                for kb in kbs:
                    if kb == 2 * qs + 1:
                        # diag end: only q-high halves live (never first)
                        s_n = p_s.tile([128, 256], F32, tag="s")
                        nc.tensor.matmul(s_n, kT_sb[:, kb * 128:(kb + 1) * 128],
                                         qT_sb[:, h2, q0 + 128:q0 + 256],
                                         start=True, stop=True)
                        pt_n = ptp.tile([128, 256], BF16, tag="pt")
                        nc.scalar.activation(pt_n, s_n, AF.Exp, scale=float(SCALE))
                        nc.vector.tensor_tensor(pt_n, pt_n, masksn_sb[:, 0, :], OP.mult)
                        nc.tensor.matmul(o_v[:, :, 128:256], vn_sb[:, kb, :], pt_n,
                                         start=False, stop=True, skip_group_check=True)
                        nc.tensor.matmul(r_v[:, :, 128:256], ones_sb, pt_n,
                                         start=False, stop=True, skip_group_check=True)
                        continue
                    if far and kb == kb0:
                        # far edge: only q-low halves live (never first)
                        s_n = p_s.tile([128, 256], F32, tag="s")
                        nc.tensor.matmul(s_n, kT_sb[:, kb * 128:(kb + 1) * 128],
                                         qT_sb[:, h2, q0:q0 + 128],
                                         start=True, stop=True)
                        pt_n = ptp.tile([128, 256], BF16, tag="pt")
                        nc.scalar.activation(pt_n, s_n, AF.Exp, scale=float(SCALE))
                        nc.vector.tensor_tensor(pt_n, pt_n, masksn_sb[:, 1, :], OP.mult)
                        nc.tensor.matmul(o_v[:, :, 0:128], vn_sb[:, kb, :], pt_n,
                                         start=False, stop=False, skip_group_check=True)
                        nc.tensor.matmul(r_v[:, :, 0:128], ones_sb, pt_n,
                                         start=False, stop=False, skip_group_check=True)
                        continue
                    s_ps = p_s.tile([128, 512], F32, tag="s")
                    nc.tensor.matmul(s_ps,
                                     kT_sb[:, kb * 128:(kb + 1) * 128],
                                     qT_sb[:, h2, q0:q0 + TC],
                                     start=True, stop=True)
                    pt = ptp.tile([128, 512], BF16, tag="pt")
                    nc.scalar.activation(pt, s_ps, AF.Exp, scale=float(SCALE))
                    mi = None
                    if kb == 2 * qs:
                        mi = 0
                    elif far and kb == kb0 + 1:
                        mi = 3
                    if mi is not None:
                        nc.vector.tensor_tensor(pt, pt, masks_sb[:, mi, :], OP.mult)
                    nc.tensor.matmul(o_ps, vn_sb[:, kb, :], pt,
                                     start=(kb == first), stop=False, skip_group_check=True)
                    nc.tensor.matmul(r_ps, ones_sb, pt,
                                     start=(kb == first), stop=False, skip_group_check=True)
                rr = wk2.tile([128, 512], F32, tag="rr")
                nc.vector.reciprocal(rr, r_ps)
                nc.vector.tensor_mul(yT_sb[:, h2, q0:q0 + TC], o_ps, rr)

            def emit_out(qs, osp):
                for os_ in (2 * osp, 2 * osp + 1):
                    for tt in (2 * qs, 2 * qs + 1):
                        pool3, tag3 = (p_s, "s") if tt % 2 == 0 else (p_or, "or")
                        po = pool3.tile([128, 512], F32, tag=tag3)
                        for h in range(4):
                            nc.tensor.matmul(po, yT_sb[:, h, tt * 128:(tt + 1) * 128],
                                             wo_sb[:, h, os_ * 512:(os_ + 1) * 512],
                                             start=(h == 0), stop=(h == 3))
                        stage = stg.tile([128, 512], BF16, tag="stage")
                        if tt % 2 == 0:
                            nc.vector.tensor_copy(stage, po)
                        else:
                            nc.scalar.copy(stage, po)
                        nc.sync.dma_start(
                            out=out[tt * 128:(tt + 1) * 128, os_ * 512:(os_ + 1) * 512],
                            in_=stage)

            for qs in range(NTC):
                emit_attn(0, qs)
                if qs >= 1:
                    emit_out(qs - 1, 0)
                emit_attn(1, qs)
                if qs >= 1:
                    emit_out(qs - 1, 1)
            emit_out(NTC - 1, 0)
            emit_out(NTC - 1, 1)

    nc.compile()
    return nc


def _hd_perm():
    """Head-dim permutation: position 32q+j holds old dim 16q+j and position
    32q+16+j holds old dim 16q+j+64, so each rope pair (d, d+64) sits at
    (p, p^16) — swappable by DVE stream_shuffle within 32-partition quarters.
    Attention/rms are invariant to any consistent q/k head-dim permutation."""
    delta = np.empty(128, np.int64)
    for q in range(4):
        for j in range(16):
            delta[32 * q + j] = 16 * q + j
            delta[32 * q + 16 + j] = 16 * q + j + 64
    return delta


def _masks():
    jj = np.arange(128)[:, None]
    ii = np.arange(128)[None, :]
    tri_d = (jj <= ii).astype(np.float32)   # diag block: keep j <= i
    tri_f = (jj >= ii).astype(np.float32)   # far block: keep j >= i - WIN
    one = np.ones((128, 128), np.float32)
    zero = np.zeros((128, 128), np.float32)
    m0 = np.concatenate([tri_d, one], 1)
    m1 = np.concatenate([zero, tri_d], 1)
    m2 = np.concatenate([tri_f, zero], 1)
    m3 = np.concatenate([one, tri_f], 1)
    base = np.ascontiguousarray(np.tile(np.stack([m0, m1, m2, m3]), (1, 1, 2)))
    # [0]: diag-end (tri_d for both heads); [1]: far-edge (tri_f for both heads)
    mn = np.stack([np.concatenate([tri_d, tri_d], 1),
                   np.concatenate([tri_f, tri_f], 1)])
    return base, np.ascontiguousarray(mn)


def kernel(**inputs):
    import ml_dtypes
    from concourse.bass_utils import run_bass_kernel_spmd

    BF = ml_dtypes.bfloat16

    if "nc" not in _CACHE:
        _CACHE["nc"] = _build_program()
    nc = _CACHE["nc"]

    x = np.asarray(inputs["x"], np.float32)
    ve = np.asarray(inputs["ve"], np.float32)
    cos = np.asarray(inputs["cos"], np.float32)
    sin = np.asarray(inputs["sin"], np.float32)
    Wq = np.asarray(inputs["Wq"], np.float32)
    Wk = np.asarray(inputs["Wk"], np.float32)
    Wv = np.asarray(inputs["Wv"], np.float32)
    Wo = np.asarray(inputs["Wo"], np.float32)
    Wg = np.asarray(inputs["Wg"], np.float32)

    delta = _hd_perm()
    crep = np.ascontiguousarray(np.concatenate([cos.T, cos.T], 0)[delta]).astype(BF)
    ssgn = np.ascontiguousarray(np.concatenate([sin.T, -sin.T], 0)[delta]).astype(BF)
    # permute q/k head dims to the rope-pair-local layout (per 128-dim head)
    Wqp = Wq.reshape(E, NH, HD)[:, :, delta].reshape(E, NH * HD)
    Wkp = Wk.reshape(E, NKV, HD)[:, :, delta].reshape(E, NKV * HD)
    masks, masksn = _masks()
    masks = masks.astype(BF)
    masksn = masksn.astype(BF)
    ones128 = np.ones((128, 128), BF)

    in_maps = []
    for c in range(8):
        b, g = divmod(c, 4)
        in_maps.append({
            "xT": np.ascontiguousarray(x[b].T).astype(BF),
            "veT": np.ascontiguousarray(ve[b, :, g * HD:(g + 1) * HD].T).astype(BF),
            "crep": crep,
            "ssgn": ssgn,
            "wq": np.ascontiguousarray(Wqp[:, g * 512:(g + 1) * 512]).astype(BF),
            "wk": np.ascontiguousarray(Wkp[:, g * HD:(g + 1) * HD]).astype(BF),
            "wv": np.ascontiguousarray(Wv[:, g * HD:(g + 1) * HD]).astype(BF),
            "wg": np.ascontiguousarray(np.repeat(Wg[:, g:g + 1], 128, 1)).astype(BF),
            "wo": np.ascontiguousarray(Wo[g * 512:(g + 1) * 512, :]).astype(BF),
            "m_in": masks,
            "mn_in": masksn,
            "ones_in": ones128,
        })

    res = run_bass_kernel_spmd(nc, in_maps, core_ids=list(range(8)))
    parts = [np.asarray(res.results[c]["out"], np.float32) for c in range(8)]
    out = np.stack([parts[0] + parts[1] + parts[2] + parts[3],
                    parts[4] + parts[5] + parts[6] + parts[7]])
    return out.astype(np.float32)


# revision 28
# speedup vs baseline: 1.0271x; 1.0271x over previous
"""Sliding-window causal GQA self-attention (B=2, T=2048, 16 q-heads, 4 kv-heads,
head_dim=128, window=1024) on 8 trn2 NeuronCores.

Sharding: core = (batch b, kv-group g) -> 4 query heads + 1 kv head, full T.
Wo is row-parallel; each core emits a [T, 2048] partial that the host sums per
batch (the unshard step for the row-parallel layout).

All matmul operands are bf16 (1 cycle/row on PE at any free size; halves DMA
traffic vs fp32); PSUM accumulation stays fp32. The whole program uses a single
activation-function table set (natural_log_exp_and_others: exp/ln/copy/square),
so no LoadActFuncSet reloads: the RMS rsqrt is computed as exp(-0.5*ln(ms+eps)).

Device dataflow:
  phase 1 (per 256-token chunk): qT/kT/vT projections (bf16), squares on ACT
           straight from PSUM, per-pair ones-matmul partition-sum, Ln+Exp rms
           factors, RoPE (half-swap via PSUM->SBUF DMA + [c;c], [s;-s] tables),
           gate sigmoid via Exp, V^T -> natural V via DMA transpose (bf16).
  phase 2: S^T = K^T.T @ Q^T per 128-key block x 256-query super (2 heads);
           ACT exp (scale fused) -> bf16; 0/1 triangle masks; PV + all-ones
           rowsum accumulated in PSUM; normalize on evacuation into yT (bf16).
           Far-edge key block computes only the live low query half.
           Phase-3 output matmuls for query-super qs-1 are interleaved between
           head-pairs to fill PE bubbles.
  phase 3 (interleaved): out[t, o] = sum_h yT_h^T @ Wo_h, Wo fully prefetched.
"""

import numpy as np

B, T, E = 2, 2048, 2048
NH, NKV, HD = 16, 4, 128
GATE_C = 32
WIN = 1024
EPS = 1e-6
NE = E // 128          # 16 contraction chunks
TC = 256               # phase-1 token chunk (= q-super width)
NTC = T // TC          # 8
NKB = T // 128         # 16 key blocks
SCALE = 1.0 / np.sqrt(HD)

_CACHE = {}


def _build_program():
    import concourse.bacc as bacc
    import concourse.mybir as mybir
    import concourse.tile as tile
    from concourse import bass_isa

    F32, BF16 = mybir.dt.float32, mybir.dt.bfloat16
    AF = mybir.ActivationFunctionType
    OP = mybir.AluOpType

    nc = bacc.Bacc("TRN2", target_bir_lowering=False, debug=False, num_devices=8)

    xT = nc.dram_tensor("xT", [E, T], BF16, kind="ExternalInput")
    veT = nc.dram_tensor("veT", [HD, T], BF16, kind="ExternalInput")
    crep = nc.dram_tensor("crep", [128, T], BF16, kind="ExternalInput")
    ssgn = nc.dram_tensor("ssgn", [128, T], BF16, kind="ExternalInput")
    wq = nc.dram_tensor("wq", [E, 512], BF16, kind="ExternalInput")
    wk = nc.dram_tensor("wk", [E, HD], BF16, kind="ExternalInput")
    wv = nc.dram_tensor("wv", [E, HD], BF16, kind="ExternalInput")
    wg = nc.dram_tensor("wg", [GATE_C, 128], BF16, kind="ExternalInput")
    wo = nc.dram_tensor("wo", [512, E], BF16, kind="ExternalInput")
    m_in = nc.dram_tensor("m_in", [4, 128, 512], BF16, kind="ExternalInput")
    mn_in = nc.dram_tensor("mn_in", [2, 128, 256], BF16, kind="ExternalInput")
    ones_in = nc.dram_tensor("ones_in", [128, 128], BF16, kind="ExternalInput")
    out = nc.dram_tensor("out", [T, E], BF16, kind="ExternalOutput")

    xT_r = xT.rearrange("(e k) t -> k e t", k=128)
    wq_r = wq.rearrange("(e k) d -> k e d", k=128)
    wk_r = wk.rearrange("(e k) d -> k e d", k=128)
    wv_r = wv.rearrange("(e k) d -> k e d", k=128)
    wo_r = wo.rearrange("(h d) o -> d h o", d=128)

    with tile.TileContext(nc) as tc:
        from contextlib import ExitStack
        with ExitStack() as ctx:
            cst = ctx.enter_context(tc.tile_pool(name="cst", bufs=1))
            wts = ctx.enter_context(tc.tile_pool(name="wts", bufs=1))
            xtp = ctx.enter_context(tc.tile_pool(name="xtp", bufs=3))
            res = ctx.enter_context(tc.tile_pool(name="res", bufs=1))
            qrp = ctx.enter_context(tc.tile_pool(name="qrp", bufs=4))
            wk1 = ctx.enter_context(tc.tile_pool(name="wk1", bufs=4))
            wk2 = ctx.enter_context(tc.tile_pool(name="wk2", bufs=2))
            ptp = ctx.enter_context(tc.tile_pool(name="ptp", bufs=4))
            stg = ctx.enter_context(tc.tile_pool(name="stg", bufs=4))
            p_q = ctx.enter_context(tc.tile_pool(name="p_q", bufs=2, space="PSUM"))
            p_s = ctx.enter_context(tc.tile_pool(name="p_s", bufs=3, space="PSUM"))
            p_or = ctx.enter_context(tc.tile_pool(name="p_or", bufs=3, space="PSUM"))

            # ---- tiny constants + chunk-0 / weight stream, round-robin by
            # e-group so the projection chains can start as data arrives ----
            ones_sb = cst.tile([128, 128], BF16, tag="ones")
            eps_sb = cst.tile([128, 1], F32, tag="eps")
            wg_sb = wts.tile([GATE_C, 128], BF16, tag="wg")
            nc.sync.dma_start(out=ones_sb, in_=ones_in[:])
            nc.vector.memset(eps_sb, EPS)
            nc.sync.dma_start(out=wg_sb, in_=wg[:])

            # Pin the act table to natural_log_exp_and_others (set 6): every
            # activation in this program (Exp, Ln, Square, Copy, Identity) is
            # in it, so the auto-insertion pass never needs another load.
            nc.scalar.add_instruction(mybir.InstLoadActFuncSet(
                name=nc.get_next_instruction_name(), ins=[], outs=[],
                act_func_set_id=6))

            # gate input first: it's tiny and gives PE work immediately
            xg_sb = cst.tile([GATE_C, T], BF16, tag="xg")
            nc.sync.dma_start(out=xg_sb, in_=xT[0:GATE_C, :])

            xt0 = xtp.tile([128, NE, TC], BF16, tag="xt")
            wq_sb = wts.tile([128, NE, 512], BF16, tag="wq")
            wk_sb = wts.tile([128, NE, HD], BF16, tag="wk")
            wv_sb = wts.tile([128, NE, HD], BF16, tag="wv")
            for e4 in range(4):
                sl = slice(e4 * 4, (e4 + 1) * 4)
                nc.sync.dma_start(out=xt0[:, sl, :], in_=xT_r[:, sl, 0:TC])
                nc.sync.dma_start(out=wk_sb[:, sl, :], in_=wk_r[:, sl, :])
                nc.sync.dma_start(out=wq_sb[:, sl, :], in_=wq_r[:, sl, :])
            for e4 in range(4):
                sl = slice(e4 * 4, (e4 + 1) * 4)
                nc.sync.dma_start(out=wv_sb[:, sl, :], in_=wv_r[:, sl, :])

            # rope/ve tables resident for the whole run
            crep_sb = cst.tile([128, T], BF16, tag="crep")
            ssgn_sb = cst.tile([128, T], BF16, tag="ssgn")
            veT_sb = cst.tile([HD, T], BF16, tag="veT")
            nc.sync.dma_start(out=crep_sb, in_=crep[:])
            nc.sync.dma_start(out=ssgn_sb, in_=ssgn[:])
            nc.sync.dma_start(out=veT_sb, in_=veT[:])

            masks_sb = cst.tile([128, 4, 512], BF16, tag="masks")
            masksn_sb = cst.tile([128, 2, 256], BF16, tag="masksn")
            wo_sb = wts.tile([128, 4, E], BF16, tag="wo")

            # ---- persistent results ----
            qT_sb = res.tile([128, 4, T], BF16, tag="qT")
            kT_sb = res.tile([128, T], BF16, tag="kT")
            yT_sb = res.tile([128, 4, T], BF16, tag="yT")
            vn_sb = res.tile([128, NKB, HD], BF16, tag="vn")

            # ================= phase 0: all gate sigmoids ==================
            # One Exp act-table period at program start; phase 1 then runs on
            # the Sqrt table only and phase 2/3 on Exp only (3 loads total).
            # g = 1/(1+exp(-u)); the 2x is folded into the v STT later.
            g_all = cst.tile([128, T], F32, tag="gall")
            for gs in range(4):
                sl = slice(gs * 512, (gs + 1) * 512)
                gp = p_s.tile([128, 512], F32, tag="s")
                nc.tensor.matmul(gp, wg_sb, xg_sb[:, sl], start=True, stop=True)
                nc.scalar.activation(g_all[:, sl], gp, AF.Exp, scale=-1.0)
                nc.vector.tensor_scalar_add(g_all[:, sl], g_all[:, sl], 1.0)
                nc.vector.reciprocal(g_all[:, sl], g_all[:, sl])

            # ================= phase 1 =====================================
            _CHUNK_XT = {}
            for tcix in range(NTC):
                ts = tcix * TC
                if tcix == 0:
                    xt = xt0
                    xt_next = None
                else:
                    xt = _CHUNK_XT[tcix]
                    xt_next = None
                if tcix + 1 < NTC:
                    xt_next = xtp.tile([128, NE, TC], BF16, tag="xt")
                    _CHUNK_XT[tcix + 1] = xt_next
                c_sl = crep_sb[:, ts:ts + TC]
                s_sl = ssgn_sb[:, ts:ts + TC]
                ve_sl = veT_sb[:, ts:ts + TC]
                g_rep = g_all[:, ts:ts + TC]

                def prefetch(part):
                    # spread next-chunk x DMA through this chunk so it never
                    # blocks latency-critical small transfers on the queue
                    if xt_next is not None:
                        sl = slice(part * 4, (part + 1) * 4)
                        nc.sync.dma_start(
                            out=xt_next[:, sl, :],
                            in_=xT_r[:, sl, (ts + TC):(ts + 2 * TC)])

                # k first so attention's S matmuls unblock as early as
                # possible; v mid-chunk so vn is ready before the head tail
                srcs = [("k", 0), ("q", 0), ("q", 1), ("q", 2), ("q", 3)]
                chunk_qraws = []
                sq_pair = None
                rr_pair = None
                ps_bank = None
                bank_n = 0
                for i, (kind, h) in enumerate(srcs):
                    if bank_n == 0:
                        ps_bank = p_q.tile([128, 512], F32, tag="q")
                    ps = ps_bank[:, bank_n * TC:(bank_n + 1) * TC]
                    bank_n = (bank_n + 1) % 2
                    w_sb = wq_sb if kind == "q" else wk_sb
                    for e in range(NE):
                        lhs = w_sb[:, e, h * 128:(h + 1) * 128] if kind == "q" else w_sb[:, e, :]
                        nc.tensor.matmul(ps, lhs, xt[:, e, :],
                                         start=(e == 0), stop=(e == NE - 1))
                    half = i % 2
                    if half == 0:
                        sq_pair = wk1.tile([128, 512], BF16, tag="sq")
                        rr_pair = wk2.tile([128, 512], F32, tag="rrms")
                    nc.scalar.activation(sq_pair[:, half * TC:(half + 1) * TC],
                                         ps, AF.Square)
                    # rotate-half partner via DVE partition shuffle (groups of
                    # 4 partitions; swapping halves is an involution so the
                    # mask direction is irrelevant)
                    qsw = wk1.tile([128, TC], F32, tag="qsw")
                    nc.vector.stream_shuffle(qsw, ps,
                                             list(range(16, 32)) + list(range(16)))
                    chunk_qraws.append((ps, qsw))
                    if half == 1 or i == 4:
                        wd = 512 if half == 1 else 256
                        ss_sb = wk2.tile([128, 512], F32, tag="ssr")
                        nc.gpsimd.partition_all_reduce(
                            ss_sb[:, 0:wd], sq_pair[:, 0:wd], channels=128,
                            reduce_op=bass_isa.ReduceOp.add)
                        lt = wk1.tile([128, 512], F32, tag="lt")
                        nc.scalar.activation(lt[:, 0:wd], ss_sb[:, 0:wd],
                                             AF.Ln, bias=eps_sb, scale=1.0 / HD)
                        nc.scalar.activation(rr_pair[:, 0:wd], lt[:, 0:wd],
                                             AF.Exp, scale=-0.5)
                        done = [i - 1, i] if half == 1 else [i]
                        for ii in done:
                            kind2, h2 = srcs[ii]
                            qraw2, qsw2 = chunk_qraws[ii]
                            rrms = rr_pair[:, (ii % 2) * TC:(ii % 2 + 1) * TC]
                            tA = wk1.tile([128, TC], F32, tag="tA")
                            tB = wk1.tile([128, TC], F32, tag="tB")
                            nc.vector.tensor_tensor(tA, qraw2, c_sl, OP.mult)
                            nc.gpsimd.tensor_tensor(tB, qsw2, s_sl, OP.mult)
                            nc.vector.tensor_add(tA, tA, tB)
                            dest = (qT_sb[:, h2, ts:ts + TC] if kind2 == "q"
                                    else kT_sb[:, ts:ts + TC])
                            nc.vector.tensor_mul(dest, tA, rrms)
                        prefetch(i // 2)

                    if i == 2:
                        # v chain mid-chunk: projection + gated ve; natural
                        # layout via DMA transpose
                        if bank_n == 0:
                            ps_bank = p_q.tile([128, 512], F32, tag="q")
                        ps_v = ps_bank[:, bank_n * TC:(bank_n + 1) * TC]
                        bank_n = (bank_n + 1) % 2
                        for e in range(NE):
                            nc.tensor.matmul(ps_v, wv_sb[:, e, :], xt[:, e, :],
                                             start=(e == 0), stop=(e == NE - 1))
                        tv = wk1.tile([128, TC], F32, tag="tA")
                        nc.gpsimd.tensor_tensor(tv, ve_sl, g_rep, OP.mult)
                        vt = wk1.tile([128, TC], BF16, tag="tB")
                        nc.vector.scalar_tensor_tensor(vt, tv, 2.0, ps_v,
                                                       OP.mult, OP.add)
                        for tb in range(TC // 128):
                            nc.sync.dma_start_transpose(
                                out=vn_sb[:, tcix * 2 + tb, :],
                                in_=vt[:, tb * 128:(tb + 1) * 128])

                prefetch(3)
                if tcix == 1:
                    # phase-2 masks: needed once attention for qs=0 hoists in
                    nc.sync.dma_start(out=masks_sb,
                                      in_=m_in.rearrange("m p f -> p m f"))
                    nc.sync.dma_start(out=masksn_sb,
                                      in_=mn_in.rearrange("m p f -> p m f"))
                if tcix == 3:
                    # full Wo prefetch (bf16, 2 MiB); first used by emit_out(0)
                    nc.sync.dma_start(out=wo_sb, in_=wo_r)

            # ============ phase 2 + interleaved phase 3 ====================
            def emit_attn(hp, qs):
                h2 = slice(2 * hp, 2 * hp + 2)
                q0 = qs * TC
                kb0 = max(0, 2 * qs - 8)
                kb1 = 2 * qs + 2
                far = qs >= 4  # far window edge exists -> kb0 is half-live
                o_ps = p_or.tile([128, 512], F32, tag="or")
                r_ps = p_or.tile([128, 512], F32, tag="or")
                o_v = o_ps.rearrange("p (h q) -> p h q", h=2)
                r_v = r_ps.rearrange("p (h q) -> p h q", h=2)
                kbs = list(range(kb0, kb1))
                if far:
                    # kb0 only touches the low query half; emit kb0+1 first so
                    # it opens (start=True) the full-width PSUM accumulation.
                    kbs[0], kbs[1] = kbs[1], kbs[0]
                first = kbs[0]
                for kb in kbs:
                    if kb == 2 * qs + 1:
                        # diag end: only q-high halves live (never first)
                        s_n = p_s.tile([128, 256], F32, tag="s")
                        nc.tensor.matmul(s_n, kT_sb[:, kb * 128:(kb + 1) * 128],
                                         qT_sb[:, h2, q0 + 128:q0 + 256],
                                         start=True, stop=True)
                        pt_n = ptp.tile([128, 256], BF16, tag="pt")
                        nc.scalar.activation(pt_n, s_n, AF.Exp, scale=float(SCALE))
                        nc.vector.tensor_tensor(pt_n, pt_n, masksn_sb[:, 0, :], OP.mult)
                        nc.tensor.matmul(o_v[:, :, 128:256], vn_sb[:, kb, :], pt_n,
                                         start=False, stop=True, skip_group_check=True)
                        nc.tensor.matmul(r_v[:, :, 128:256], ones_sb, pt_n,
                                         start=False, stop=True, skip_group_check=True)
                        continue
                    if far and kb == kb0:
                        # far edge: only q-low halves live (never first)
                        s_n = p_s.tile([128, 256], F32, tag="s")
                        nc.tensor.matmul(s_n, kT_sb[:, kb * 128:(kb + 1) * 128],
                                         qT_sb[:, h2, q0:q0 + 128],
                                         start=True, stop=True)
                        pt_n = ptp.tile([128, 256], BF16, tag="pt")
                        nc.scalar.activation(pt_n, s_n, AF.Exp, scale=float(SCALE))
                        nc.vector.tensor_tensor(pt_n, pt_n, masksn_sb[:, 1, :], OP.mult)
                        nc.tensor.matmul(o_v[:, :, 0:128], vn_sb[:, kb, :], pt_n,
                                         start=False, stop=False, skip_group_check=True)
                        nc.tensor.matmul(r_v[:, :, 0:128], ones_sb, pt_n,
                                         start=False, stop=False, skip_group_check=True)
                        continue
                    s_ps = p_s.tile([128, 512], F32, tag="s")
                    nc.tensor.matmul(s_ps,
                                     kT_sb[:, kb * 128:(kb + 1) * 128],
                                     qT_sb[:, h2, q0:q0 + TC],
                                     start=True, stop=True)
                    pt = ptp.tile([128, 512], BF16, tag="pt")
                    nc.scalar.activation(pt, s_ps, AF.Exp, scale=float(SCALE))
                    mi = None
                    if kb == 2 * qs:
                        mi = 0
                    elif far and kb == kb0 + 1:
                        mi = 3
                    if mi is not None:
                        nc.vector.tensor_tensor(pt, pt, masks_sb[:, mi, :], OP.mult)
                    nc.tensor.matmul(o_ps, vn_sb[:, kb, :], pt,
                                     start=(kb == first), stop=False, skip_group_check=True)
                    nc.tensor.matmul(r_ps, ones_sb, pt,
                                     start=(kb == first), stop=False, skip_group_check=True)
                rr = wk2.tile([128, 512], F32, tag="rr")
                nc.vector.reciprocal(rr, r_ps)
                nc.vector.tensor_mul(yT_sb[:, h2, q0:q0 + TC], o_ps, rr)

            def emit_out(qs, osp):
                for os_ in (2 * osp, 2 * osp + 1):
                    for tt in (2 * qs, 2 * qs + 1):
                        pool3, tag3 = (p_s, "s") if tt % 2 == 0 else (p_or, "or")
                        po = pool3.tile([128, 512], F32, tag=tag3)
                        for h in range(4):
                            nc.tensor.matmul(po, yT_sb[:, h, tt * 128:(tt + 1) * 128],
                                             wo_sb[:, h, os_ * 512:(os_ + 1) * 512],
                                             start=(h == 0), stop=(h == 3))
                        stage = stg.tile([128, 512], BF16, tag="stage")
                        if tt % 2 == 0:
                            nc.vector.tensor_copy(stage, po)
                        else:
                            nc.scalar.copy(stage, po)
                        nc.sync.dma_start(
                            out=out[tt * 128:(tt + 1) * 128, os_ * 512:(os_ + 1) * 512],
                            in_=stage)

            for qs in range(NTC):
                emit_attn(0, qs)
                if qs >= 1:
                    emit_out(qs - 1, 0)
                emit_attn(1, qs)
                if qs >= 1:
                    emit_out(qs - 1, 1)
            emit_out(NTC - 1, 0)
            emit_out(NTC - 1, 1)

    nc.compile()
    return nc


def _hd_perm():
    """Head-dim permutation: position 32q+j holds old dim 16q+j and position
    32q+16+j holds old dim 16q+j+64, so each rope pair (d, d+64) sits at
    (p, p^16) — swappable by DVE stream_shuffle within 32-partition quarters.
    Attention/rms are invariant to any consistent q/k head-dim permutation."""
    delta = np.empty(128, np.int64)
    for q in range(4):
        for j in range(16):
            delta[32 * q + j] = 16 * q + j
            delta[32 * q + 16 + j] = 16 * q + j + 64
    return delta


def _masks():
    jj = np.arange(128)[:, None]
    ii = np.arange(128)[None, :]
    tri_d = (jj <= ii).astype(np.float32)   # diag block: keep j <= i
    tri_f = (jj >= ii).astype(np.float32)   # far block: keep j >= i - WIN
    one = np.ones((128, 128), np.float32)
    zero = np.zeros((128, 128), np.float32)
    m0 = np.concatenate([tri_d, one], 1)
    m1 = np.concatenate([zero, tri_d], 1)
    m2 = np.concatenate([tri_f, zero], 1)
    m3 = np.concatenate([one, tri_f], 1)
    base = np.ascontiguousarray(np.tile(np.stack([m0, m1, m2, m3]), (1, 1, 2)))
    # [0]: diag-end (tri_d for both heads); [1]: far-edge (tri_f for both heads)
    mn = np.stack([np.concatenate([tri_d, tri_d], 1),
                   np.concatenate([tri_f, tri_f], 1)])
    return base, np.ascontiguousarray(mn)


def kernel(**inputs):
    import ml_dtypes
    from concourse.bass_utils import run_bass_kernel_spmd

    BF = ml_dtypes.bfloat16

    if "nc" not in _CACHE:
        _CACHE["nc"] = _build_program()
    nc = _CACHE["nc"]

    x = np.asarray(inputs["x"], np.float32)
    ve = np.asarray(inputs["ve"], np.float32)
    cos = np.asarray(inputs["cos"], np.float32)
    sin = np.asarray(inputs["sin"], np.float32)
    Wq = np.asarray(inputs["Wq"], np.float32)
    Wk = np.asarray(inputs["Wk"], np.float32)
    Wv = np.asarray(inputs["Wv"], np.float32)
    Wo = np.asarray(inputs["Wo"], np.float32)
    Wg = np.asarray(inputs["Wg"], np.float32)

    delta = _hd_perm()
    crep = np.ascontiguousarray(np.concatenate([cos.T, cos.T], 0)[delta]).astype(BF)
    ssgn = np.ascontiguousarray(np.concatenate([sin.T, -sin.T], 0)[delta]).astype(BF)
    # permute q/k head dims to the rope-pair-local layout (per 128-dim head)
    Wqp = Wq.reshape(E, NH, HD)[:, :, delta].reshape(E, NH * HD)
    Wkp = Wk.reshape(E, NKV, HD)[:, :, delta].reshape(E, NKV * HD)
    masks, masksn = _masks()
    masks = masks.astype(BF)
    masksn = masksn.astype(BF)
    ones128 = np.ones((128, 128), BF)

    in_maps = []
    for c in range(8):
        b, g = divmod(c, 4)
        in_maps.append({
            "xT": np.ascontiguousarray(x[b].T).astype(BF),
            "veT": np.ascontiguousarray(ve[b, :, g * HD:(g + 1) * HD].T).astype(BF),
            "crep": crep,
            "ssgn": ssgn,
            "wq": np.ascontiguousarray(Wqp[:, g * 512:(g + 1) * 512]).astype(BF),
            "wk": np.ascontiguousarray(Wkp[:, g * HD:(g + 1) * HD]).astype(BF),
            "wv": np.ascontiguousarray(Wv[:, g * HD:(g + 1) * HD]).astype(BF),
            "wg": np.ascontiguousarray(np.repeat(Wg[:, g:g + 1], 128, 1)).astype(BF),
            "wo": np.ascontiguousarray(Wo[g * 512:(g + 1) * 512, :]).astype(BF),
            "m_in": masks,
            "mn_in": masksn,
            "ones_in": ones128,
        })

    res = run_bass_kernel_spmd(nc, in_maps, core_ids=list(range(8)))
    parts = [np.asarray(res.results[c]["out"], np.float32) for c in range(8)]
    out = np.stack([parts[0] + parts[1] + parts[2] + parts[3],
                    parts[4] + parts[5] + parts[6] + parts[7]])
    return out.astype(np.float32)


# revision 29
# speedup vs baseline: 1.0571x; 1.0292x over previous
"""Sliding-window causal GQA self-attention (B=2, T=2048, 16 q-heads, 4 kv-heads,
head_dim=128, window=1024) on 8 trn2 NeuronCores.

Sharding: core = (batch b, kv-group g) -> 4 query heads + 1 kv head, full T.
Wo is row-parallel; each core emits a [T, 2048] partial that the host sums per
batch (the unshard step for the row-parallel layout).

All matmul operands are bf16 (1 cycle/row on PE at any free size; halves DMA
traffic vs fp32); PSUM accumulation stays fp32. The whole program uses a single
activation-function table set (natural_log_exp_and_others: exp/ln/copy/square),
so no LoadActFuncSet reloads: the RMS rsqrt is computed as exp(-0.5*ln(ms+eps)).

Device dataflow:
  phase 1 (per 256-token chunk): qT/kT/vT projections (bf16), squares on ACT
           straight from PSUM, per-pair ones-matmul partition-sum, Ln+Exp rms
           factors, RoPE (half-swap via PSUM->SBUF DMA + [c;c], [s;-s] tables),
           gate sigmoid via Exp, V^T -> natural V via DMA transpose (bf16).
  phase 2: S^T = K^T.T @ Q^T per 128-key block x 256-query super (2 heads);
           ACT exp (scale fused) -> bf16; 0/1 triangle masks; PV + all-ones
           rowsum accumulated in PSUM; normalize on evacuation into yT (bf16).
           Far-edge key block computes only the live low query half.
           Phase-3 output matmuls for query-super qs-1 are interleaved between
           head-pairs to fill PE bubbles.
  phase 3 (interleaved): out[t, o] = sum_h yT_h^T @ Wo_h, Wo fully prefetched.
"""

import numpy as np

B, T, E = 2, 2048, 2048
NH, NKV, HD = 16, 4, 128
GATE_C = 32
WIN = 1024
EPS = 1e-6
NE = E // 128          # 16 contraction chunks
TC = 256               # phase-1 token chunk (= q-super width)
NTC = T // TC          # 8
NKB = T // 128         # 16 key blocks
SCALE = 1.0 / np.sqrt(HD)

_CACHE = {}


def _build_program():
    import concourse.bacc as bacc
    import concourse.mybir as mybir
    import concourse.tile as tile
    from concourse import bass_isa

    F32, BF16 = mybir.dt.float32, mybir.dt.bfloat16
    AF = mybir.ActivationFunctionType
    OP = mybir.AluOpType

    nc = bacc.Bacc("TRN2", target_bir_lowering=False, debug=False, num_devices=8)

    xT = nc.dram_tensor("xT", [E, T], BF16, kind="ExternalInput")
    veT = nc.dram_tensor("veT", [HD, T], BF16, kind="ExternalInput")
    crep = nc.dram_tensor("crep", [128, T], BF16, kind="ExternalInput")
    ssgn = nc.dram_tensor("ssgn", [128, T], BF16, kind="ExternalInput")
    wq = nc.dram_tensor("wq", [E, 512], BF16, kind="ExternalInput")
    wk = nc.dram_tensor("wk", [E, HD], BF16, kind="ExternalInput")
    wv = nc.dram_tensor("wv", [E, HD], BF16, kind="ExternalInput")
    wg = nc.dram_tensor("wg", [GATE_C, 128], BF16, kind="ExternalInput")
    wo = nc.dram_tensor("wo", [512, E], BF16, kind="ExternalInput")
    m_in = nc.dram_tensor("m_in", [4, 128, 512], BF16, kind="ExternalInput")
    mn_in = nc.dram_tensor("mn_in", [2, 128, 256], BF16, kind="ExternalInput")
    ones_in = nc.dram_tensor("ones_in", [128, 128], BF16, kind="ExternalInput")
    out = nc.dram_tensor("out", [T, E], BF16, kind="ExternalOutput")

    xT_r = xT.rearrange("(e k) t -> k e t", k=128)
    wq_r = wq.rearrange("(e k) d -> k e d", k=128)
    wk_r = wk.rearrange("(e k) d -> k e d", k=128)
    wv_r = wv.rearrange("(e k) d -> k e d", k=128)
    wo_r = wo.rearrange("(h d) o -> d h o", d=128)

    with tile.TileContext(nc) as tc:
        from contextlib import ExitStack
        with ExitStack() as ctx:
            cst = ctx.enter_context(tc.tile_pool(name="cst", bufs=1))
            wts = ctx.enter_context(tc.tile_pool(name="wts", bufs=1))
            xtp = ctx.enter_context(tc.tile_pool(name="xtp", bufs=3))
            res = ctx.enter_context(tc.tile_pool(name="res", bufs=1))
            qrp = ctx.enter_context(tc.tile_pool(name="qrp", bufs=4))
            wk1 = ctx.enter_context(tc.tile_pool(name="wk1", bufs=4))
            wk2 = ctx.enter_context(tc.tile_pool(name="wk2", bufs=2))
            ptp = ctx.enter_context(tc.tile_pool(name="ptp", bufs=4))
            stg = ctx.enter_context(tc.tile_pool(name="stg", bufs=4))
            p_q = ctx.enter_context(tc.tile_pool(name="p_q", bufs=3, space="PSUM"))
            p_s = ctx.enter_context(tc.tile_pool(name="p_s", bufs=3, space="PSUM"))
            p_or = ctx.enter_context(tc.tile_pool(name="p_or", bufs=2, space="PSUM"))

            # ---- tiny constants + chunk-0 / weight stream, round-robin by
            # e-group so the projection chains can start as data arrives ----
            ones_sb = cst.tile([128, 128], BF16, tag="ones")
            eps_sb = cst.tile([128, 1], F32, tag="eps")
            wg_sb = wts.tile([GATE_C, 128], BF16, tag="wg")
            nc.sync.dma_start(out=ones_sb, in_=ones_in[:])
            nc.vector.memset(eps_sb, EPS)
            nc.sync.dma_start(out=wg_sb, in_=wg[:])

            # Pin the act table to natural_log_exp_and_others (set 6): every
            # activation in this program (Exp, Ln, Square, Copy, Identity) is
            # in it, so the auto-insertion pass never needs another load.
            nc.scalar.add_instruction(mybir.InstLoadActFuncSet(
                name=nc.get_next_instruction_name(), ins=[], outs=[],
                act_func_set_id=6))

            # gate input first: it's tiny and gives PE work immediately
            xg_sb = cst.tile([GATE_C, T], BF16, tag="xg")
            nc.sync.dma_start(out=xg_sb, in_=xT[0:GATE_C, :])

            xt0 = xtp.tile([128, NE, TC], BF16, tag="xt")
            wq_sb = wts.tile([128, NE, 512], BF16, tag="wq")
            wk_sb = wts.tile([128, NE, HD], BF16, tag="wk")
            wv_sb = wts.tile([128, NE, HD], BF16, tag="wv")
            for e4 in range(4):
                sl = slice(e4 * 4, (e4 + 1) * 4)
                nc.sync.dma_start(out=xt0[:, sl, :], in_=xT_r[:, sl, 0:TC])
                nc.sync.dma_start(out=wk_sb[:, sl, :], in_=wk_r[:, sl, :])
                nc.sync.dma_start(out=wq_sb[:, sl, :], in_=wq_r[:, sl, :])
            for e4 in range(4):
                sl = slice(e4 * 4, (e4 + 1) * 4)
                nc.sync.dma_start(out=wv_sb[:, sl, :], in_=wv_r[:, sl, :])

            # rope/ve tables resident for the whole run
            crep_sb = cst.tile([128, T], BF16, tag="crep")
            ssgn_sb = cst.tile([128, T], BF16, tag="ssgn")
            veT_sb = cst.tile([HD, T], BF16, tag="veT")
            nc.sync.dma_start(out=crep_sb, in_=crep[:])
            nc.sync.dma_start(out=ssgn_sb, in_=ssgn[:])
            nc.sync.dma_start(out=veT_sb, in_=veT[:])

            masks_sb = cst.tile([128, 4, 512], BF16, tag="masks")
            masksn_sb = cst.tile([128, 2, 256], BF16, tag="masksn")
            wo_sb = wts.tile([128, 4, E], BF16, tag="wo")

            # ---- persistent results ----
            qT_sb = res.tile([128, 4, T], BF16, tag="qT")
            kT_sb = res.tile([128, T], BF16, tag="kT")
            yT_sb = res.tile([128, 4, T], BF16, tag="yT")
            vn_sb = res.tile([128, NKB, HD], BF16, tag="vn")

            # ================= phase 0: all gate sigmoids ==================
            # One Exp act-table period at program start; phase 1 then runs on
            # the Sqrt table only and phase 2/3 on Exp only (3 loads total).
            # g = 1/(1+exp(-u)); the 2x is folded into the v STT later.
            g_all = cst.tile([128, T], F32, tag="gall")
            for gs in range(4):
                sl = slice(gs * 512, (gs + 1) * 512)
                gp = p_s.tile([128, 512], F32, tag="s")
                nc.tensor.matmul(gp, wg_sb, xg_sb[:, sl], start=True, stop=True)
                nc.scalar.activation(g_all[:, sl], gp, AF.Exp, scale=-1.0)
                nc.vector.tensor_scalar_add(g_all[:, sl], g_all[:, sl], 1.0)
                nc.vector.reciprocal(g_all[:, sl], g_all[:, sl])

            # ================= phase 1 =====================================
            _CHUNK_XT = {}
            for tcix in range(NTC):
                ts = tcix * TC
                if tcix == 0:
                    xt = xt0
                    xt_next = None
                else:
                    xt = _CHUNK_XT[tcix]
                    xt_next = None
                if tcix + 1 < NTC:
                    xt_next = xtp.tile([128, NE, TC], BF16, tag="xt")
                    _CHUNK_XT[tcix + 1] = xt_next
                c_sl = crep_sb[:, ts:ts + TC]
                s_sl = ssgn_sb[:, ts:ts + TC]
                ve_sl = veT_sb[:, ts:ts + TC]
                g_rep = g_all[:, ts:ts + TC]

                def prefetch(part):
                    # spread next-chunk x DMA through this chunk so it never
                    # blocks latency-critical small transfers on the queue
                    if xt_next is not None:
                        sl = slice(part * 4, (part + 1) * 4)
                        nc.sync.dma_start(
                            out=xt_next[:, sl, :],
                            in_=xT_r[:, sl, (ts + TC):(ts + 2 * TC)])

                # k first so attention's S matmuls unblock as early as
                # possible; v mid-chunk so vn is ready before the head tail
                srcs = [("k", 0), ("q", 0), ("q", 1), ("q", 2), ("q", 3)]
                chunk_qraws = []
                sq_pair = None
                rr_pair = None
                for i, (kind, h) in enumerate(srcs):
                    ps = p_q.tile([128, TC], F32, tag="q")
                    w_sb = wq_sb if kind == "q" else wk_sb
                    for e in range(NE):
                        lhs = w_sb[:, e, h * 128:(h + 1) * 128] if kind == "q" else w_sb[:, e, :]
                        nc.tensor.matmul(ps, lhs, xt[:, e, :],
                                         start=(e == 0), stop=(e == NE - 1))
                    half = i % 2
                    if half == 0:
                        sq_pair = wk1.tile([128, 512], BF16, tag="sq")
                        rr_pair = wk2.tile([128, 512], F32, tag="rrms")
                    nc.scalar.activation(sq_pair[:, half * TC:(half + 1) * TC],
                                         ps, AF.Square)
                    # rotate-half partner via DVE partition shuffle (groups of
                    # 4 partitions; swapping halves is an involution so the
                    # mask direction is irrelevant)
                    qsw = wk1.tile([128, TC], F32, tag="qsw")
                    nc.vector.stream_shuffle(qsw, ps,
                                             list(range(16, 32)) + list(range(16)))
                    chunk_qraws.append((ps, qsw))
                    if half == 1 or i == 4:
                        wd = 512 if half == 1 else 256
                        ss_sb = wk2.tile([128, 512], F32, tag="ssr")
                        nc.gpsimd.partition_all_reduce(
                            ss_sb[:, 0:wd], sq_pair[:, 0:wd], channels=128,
                            reduce_op=bass_isa.ReduceOp.add)
                        lt = wk1.tile([128, 512], F32, tag="lt")
                        nc.scalar.activation(lt[:, 0:wd], ss_sb[:, 0:wd],
                                             AF.Ln, bias=eps_sb, scale=1.0 / HD)
                        nc.scalar.activation(rr_pair[:, 0:wd], lt[:, 0:wd],
                                             AF.Exp, scale=-0.5)
                        done = [i - 1, i] if half == 1 else [i]
                        for ii in done:
                            kind2, h2 = srcs[ii]
                            qraw2, qsw2 = chunk_qraws[ii]
                            rrms = rr_pair[:, (ii % 2) * TC:(ii % 2 + 1) * TC]
                            tA = wk1.tile([128, TC], F32, tag="tA")
                            tB = wk1.tile([128, TC], F32, tag="tB")
                            nc.vector.tensor_tensor(tA, qraw2, c_sl, OP.mult)
                            nc.gpsimd.tensor_tensor(tB, qsw2, s_sl, OP.mult)
                            nc.vector.tensor_add(tA, tA, tB)
                            dest = (qT_sb[:, h2, ts:ts + TC] if kind2 == "q"
                                    else kT_sb[:, ts:ts + TC])
                            nc.vector.tensor_mul(dest, tA, rrms)
                        prefetch(i // 2)

                    if i == 2:
                        # v chain mid-chunk: projection + gated ve; natural
                        # layout via DMA transpose
                        ps_v = p_q.tile([128, TC], F32, tag="q")
                        for e in range(NE):
                            nc.tensor.matmul(ps_v, wv_sb[:, e, :], xt[:, e, :],
                                             start=(e == 0), stop=(e == NE - 1))
                        tv = wk1.tile([128, TC], F32, tag="tA")
                        nc.gpsimd.tensor_tensor(tv, ve_sl, g_rep, OP.mult)
                        vt = wk1.tile([128, TC], BF16, tag="tB")
                        nc.vector.scalar_tensor_tensor(vt, tv, 2.0, ps_v,
                                                       OP.mult, OP.add)
                        for tb in range(TC // 128):
                            nc.sync.dma_start_transpose(
                                out=vn_sb[:, tcix * 2 + tb, :],
                                in_=vt[:, tb * 128:(tb + 1) * 128])

                prefetch(3)
                if tcix == 1:
                    # phase-2 masks: needed once attention for qs=0 hoists in
                    nc.sync.dma_start(out=masks_sb,
                                      in_=m_in.rearrange("m p f -> p m f"))
                    nc.sync.dma_start(out=masksn_sb,
                                      in_=mn_in.rearrange("m p f -> p m f"))
                if tcix == 3:
                    # full Wo prefetch (bf16, 2 MiB); first used by emit_out(0)
                    nc.sync.dma_start(out=wo_sb, in_=wo_r)

            # ============ phase 2 + interleaved phase 3 ====================
            def emit_attn(hp, qs):
                h2 = slice(2 * hp, 2 * hp + 2)
                q0 = qs * TC
                kb0 = max(0, 2 * qs - 8)
                kb1 = 2 * qs + 2
                far = qs >= 4  # far window edge exists -> kb0 is half-live
                o_ps = p_or.tile([128, 512], F32, tag="or")
                r_ps = p_or.tile([128, 512], F32, tag="or")
                o_v = o_ps.rearrange("p (h q) -> p h q", h=2)
                r_v = r_ps.rearrange("p (h q) -> p h q", h=2)
                kbs = list(range(kb0, kb1))
                if far:
                    # kb0 only touches the low query half; emit kb0+1 first so
                    # it opens (start=True) the full-width PSUM accumulation.
                    kbs[0], kbs[1] = kbs[1], kbs[0]
                first = kbs[0]
                for kb in kbs:
                    if kb == 2 * qs + 1:
                        # diag end: only q-high halves live (never first)
                        s_n = p_s.tile([128, 256], F32, tag="s")
                        nc.tensor.matmul(s_n, kT_sb[:, kb * 128:(kb + 1) * 128],
                                         qT_sb[:, h2, q0 + 128:q0 + 256],
                                         start=True, stop=True)
                        pt_n = ptp.tile([128, 256], BF16, tag="pt")
                        nc.scalar.activation(pt_n, s_n, AF.Exp, scale=float(SCALE))
                        nc.vector.tensor_tensor(pt_n, pt_n, masksn_sb[:, 0, :], OP.mult)
                        nc.tensor.matmul(o_v[:, :, 128:256], vn_sb[:, kb, :], pt_n,
                                         start=False, stop=True, skip_group_check=True)
                        nc.tensor.matmul(r_v[:, :, 128:256], ones_sb, pt_n,
                                         start=False, stop=True, skip_group_check=True)
                        continue
                    if far and kb == kb0:
                        # far edge: only q-low halves live (never first)
                        s_n = p_s.tile([128, 256], F32, tag="s")
                        nc.tensor.matmul(s_n, kT_sb[:, kb * 128:(kb + 1) * 128],
                                         qT_sb[:, h2, q0:q0 + 128],
                                         start=True, stop=True)
                        pt_n = ptp.tile([128, 256], BF16, tag="pt")
                        nc.scalar.activation(pt_n, s_n, AF.Exp, scale=float(SCALE))
                        nc.vector.tensor_tensor(pt_n, pt_n, masksn_sb[:, 1, :], OP.mult)
                        nc.tensor.matmul(o_v[:, :, 0:128], vn_sb[:, kb, :], pt_n,
                                         start=False, stop=False, skip_group_check=True)
                        nc.tensor.matmul(r_v[:, :, 0:128], ones_sb, pt_n,
                                         start=False, stop=False, skip_group_check=True)
                        continue
                    s_ps = p_s.tile([128, 512], F32, tag="s")
                    nc.tensor.matmul(s_ps,
                                     kT_sb[:, kb * 128:(kb + 1) * 128],
                                     qT_sb[:, h2, q0:q0 + TC],
                                     start=True, stop=True)
                    pt = ptp.tile([128, 512], BF16, tag="pt")
                    nc.scalar.activation(pt, s_ps, AF.Exp, scale=float(SCALE))
                    mi = None
                    if kb == 2 * qs:
                        mi = 0
                    elif far and kb == kb0 + 1:
                        mi = 3
                    if mi is not None:
                        nc.vector.tensor_tensor(pt, pt, masks_sb[:, mi, :], OP.mult)
                    nc.tensor.matmul(o_ps, vn_sb[:, kb, :], pt,
                                     start=(kb == first), stop=False, skip_group_check=True)
                    nc.tensor.matmul(r_ps, ones_sb, pt,
                                     start=(kb == first), stop=False, skip_group_check=True)
                rr = wk2.tile([128, 512], F32, tag="rr")
                nc.vector.reciprocal(rr, r_ps)
                nc.vector.tensor_mul(yT_sb[:, h2, q0:q0 + TC], o_ps, rr)

            def emit_out(qs, osp):
                for os_ in (2 * osp, 2 * osp + 1):
                    for tt in (2 * qs, 2 * qs + 1):
                        pool3, tag3 = (p_s, "s") if tt % 2 == 0 else (p_or, "or")
                        po = pool3.tile([128, 512], F32, tag=tag3)
                        for h in range(4):
                            nc.tensor.matmul(po, yT_sb[:, h, tt * 128:(tt + 1) * 128],
                                             wo_sb[:, h, os_ * 512:(os_ + 1) * 512],
                                             start=(h == 0), stop=(h == 3))
                        stage = stg.tile([128, 512], BF16, tag="stage")
                        if tt % 2 == 0:
                            nc.vector.tensor_copy(stage, po)
                        else:
                            nc.scalar.copy(stage, po)
                        nc.sync.dma_start(
                            out=out[tt * 128:(tt + 1) * 128, os_ * 512:(os_ + 1) * 512],
                            in_=stage)

            for qs in range(NTC):
                emit_attn(0, qs)
                if qs >= 1:
                    emit_out(qs - 1, 0)
                emit_attn(1, qs)
                if qs >= 1:
                    emit_out(qs - 1, 1)
            emit_out(NTC - 1, 0)
            emit_out(NTC - 1, 1)

    nc.compile()
    return nc


def _hd_perm():
    """Head-dim permutation: position 32q+j holds old dim 16q+j and position
    32q+16+j holds old dim 16q+j+64, so each rope pair (d, d+64) sits at
    (p, p^16) — swappable by DVE stream_shuffle within 32-partition quarters.
    Attention/rms are invariant to any consistent q/k head-dim permutation."""
    delta = np.empty(128, np.int64)
    for q in range(4):
        for j in range(16):
            delta[32 * q + j] = 16 * q + j
            delta[32 * q + 16 + j] = 16 * q + j + 64
    return delta


def _masks():
    jj = np.arange(128)[:, None]
    ii = np.arange(128)[None, :]
    tri_d = (jj <= ii).astype(np.float32)   # diag block: keep j <= i
    tri_f = (jj >= ii).astype(np.float32)   # far block: keep j >= i - WIN
    one = np.ones((128, 128), np.float32)
    zero = np.zeros((128, 128), np.float32)
    m0 = np.concatenate([tri_d, one], 1)
    m1 = np.concatenate([zero, tri_d], 1)
    m2 = np.concatenate([tri_f, zero], 1)
    m3 = np.concatenate([one, tri_f], 1)
    base = np.ascontiguousarray(np.tile(np.stack([m0, m1, m2, m3]), (1, 1, 2)))
    # [0]: diag-end (tri_d for both heads); [1]: far-edge (tri_f for both heads)
    mn = np.stack([np.concatenate([tri_d, tri_d], 1),
                   np.concatenate([tri_f, tri_f], 1)])
    return base, np.ascontiguousarray(mn)


def kernel(**inputs):
    import ml_dtypes
    from concourse.bass_utils import run_bass_kernel_spmd

    BF = ml_dtypes.bfloat16

    if "nc" not in _CACHE:
        _CACHE["nc"] = _build_program()
    nc = _CACHE["nc"]

    x = np.asarray(inputs["x"], np.float32)
    ve = np.asarray(inputs["ve"], np.float32)
    cos = np.asarray(inputs["cos"], np.float32)
    sin = np.asarray(inputs["sin"], np.float32)
    Wq = np.asarray(inputs["Wq"], np.float32)
    Wk = np.asarray(inputs["Wk"], np.float32)
    Wv = np.asarray(inputs["Wv"], np.float32)
    Wo = np.asarray(inputs["Wo"], np.float32)
    Wg = np.asarray(inputs["Wg"], np.float32)

    delta = _hd_perm()
    crep = np.ascontiguousarray(np.concatenate([cos.T, cos.T], 0)[delta]).astype(BF)
    ssgn = np.ascontiguousarray(np.concatenate([sin.T, -sin.T], 0)[delta]).astype(BF)
    # permute q/k head dims to the rope-pair-local layout (per 128-dim head)
    Wqp = Wq.reshape(E, NH, HD)[:, :, delta].reshape(E, NH * HD)
    Wkp = Wk.reshape(E, NKV, HD)[:, :, delta].reshape(E, NKV * HD)
    masks, masksn = _masks()
    masks = masks.astype(BF)
    masksn = masksn.astype(BF)
    ones128 = np.ones((128, 128), BF)

    in_maps = []
    for c in range(8):
        b, g = divmod(c, 4)
        in_maps.append({
            "xT": np.ascontiguousarray(x[b].T).astype(BF),
            "veT": np.ascontiguousarray(ve[b, :, g * HD:(g + 1) * HD].T).astype(BF),
            "crep": crep,
            "ssgn": ssgn,
            "wq": np.ascontiguousarray(Wqp[:, g * 512:(g + 1) * 512]).astype(BF),
            "wk": np.ascontiguousarray(Wkp[:, g * HD:(g + 1) * HD]).astype(BF),
            "wv": np.ascontiguousarray(Wv[:, g * HD:(g + 1) * HD]).astype(BF),
            "wg": np.ascontiguousarray(np.repeat(Wg[:, g:g + 1], 128, 1)).astype(BF),
            "wo": np.ascontiguousarray(Wo[g * 512:(g + 1) * 512, :]).astype(BF),
            "m_in": masks,
            "mn_in": masksn,
            "ones_in": ones128,
        })

    res = run_bass_kernel_spmd(nc, in_maps, core_ids=list(range(8)))
    parts = [np.asarray(res.results[c]["out"], np.float32) for c in range(8)]
    out = np.stack([parts[0] + parts[1] + parts[2] + parts[3],
                    parts[4] + parts[5] + parts[6] + parts[7]])
    return out.astype(np.float32)
